# revision 9
# baseline (speedup 1.0000x reference)
"""Trainium2 Bass kernel: batched American-put binomial tree (n=256).

Math (matches the reference):
    v0_j = relu(k - s_term_j),  s_term_j = S0*exp(sig*sqrt(dt)*(2j - n))
    step t (t = 0..n-1):
        cont_j = w0*v_j + w1*v_{j+1}
        pay_j  = k - c^t * s_base_j,   c = exp(sig*sqrt(dt))
        v'_j   = max(cont_j, pay_j)
    answer = v[0] after n steps.

Mapping (pure data parallel, 1024 strikes/core = 128 partitions x 8
groups, batch on partitions, tree on the free dim; per step two DVE
scalar_tensor_tensor ops over all groups):
    U = V[:,:,a+1:b+1]*(w1/w0) + V[:,:,a:b]          # cont/w0
    V = max(U*w0, PAY[t&1][:,:,m+a:m+b])             # m = t>>1

Key tricks:
  * SORTED-BLOCK SHIFTED LAYOUT: strikes are globally sorted and cut
    into 64 blocks of 128 (one block per core/group pair). Each block
    stores its tree shifted right by s_b columns (i = j + s_b). The
    exercise boundary and the nonzero-support cap both translate with
    ln k, so per-block shifts (chosen by convex coordinate descent to
    minimize total window width) align every block's active band, and
    one common narrow window [A_t, B_t) serves the whole batch. The
    shifts live in the DATA (host-built v0/payoff tiles), so the
    program is SPMD-uniform across cores; the root lands at column s_b,
    read back from a (s_max+1)-wide output slice.
  * -INF PREFIX: columns left of the provable exercise prefix hold
    -inf; the step's max() then writes pay there exactly, so the
    shrinking exercise strip needs no copy instructions at all.
  * SLIDING PAYOFFS: pay_{t+2,j} = pay_{t,j+1}, so two static parity
    tiles read at offset t>>1 provide every step's payoff for free.
  * Window top: columns >= cap_b + s_b are exactly 0 forever (terminal
    value 0, payoff < 0), so the stale never-recomputed top column is
    still correct; columns beyond n-t+s_b can hold junk harmlessly
    (their influence cone never reaches the root).

Engine notes (measured on this silicon): fp32 DVE scalar_tensor_tensor
runs 1 elem/cycle/partition with ~180ns/op overhead; fp16 gives NO
speedup for 2-src DVE ops (884 vs 872ns at FD=640); the Pool engine's
elementwise ops crash the walrus backend; PE fp32 matmul is
quarter-rate (863ns @ FD=512) so a PE+ACT relu pipeline loses to the
DVE. Hence: fp32, all work on the DVE, minimize elements + op count.
"""

import os
import sys

for _p in ("/opt/trn_rl_repo", "/root/.axon_site/_ro/trn_rl_repo"):
    if os.path.isdir(_p) and _p not in sys.path:
        sys.path.insert(0, _p)

import numpy as np

N = 256
S0 = 100.0
SIG = 0.2
R = 0.05
DT = 1.0 / N
SQRT_DT = float(np.sqrt(DT))
U_ = float(np.exp(SIG * SQRT_DT))
D_ = float(np.exp(-SIG * SQRT_DT))
W0C = float((np.exp(-R * DT) * U_ - 1.0) / (U_ - D_))
W1C = float((1.0 - np.exp(-R * DT) * D_) / (U_ - D_))
RRATIO = W1C / W0C
C_ = U_

NCORES = 8
B = 8192
PB = B // NCORES
NPART = 128
NG = PB // NPART          # 8 groups (blocks) per core
NBLK = NCORES * NG        # 64 blocks

_J_TERM = np.arange(N + 1, dtype=np.float64)
_S_TERM = S0 * np.exp(SIG * SQRT_DT * (2.0 * _J_TERM - N))


def _s_base(j):
    return S0 * np.exp(SIG * SQRT_DT * (2.0 * np.asarray(j, np.float64)
                                        - (N - 1)))


def _block_lo(kmin_b: np.ndarray, cap_b: np.ndarray, safety: int = 2):
    """Exercise-prefix schedule lo_b[t] per block: exact f64 recursion on
    the block's smallest strike (the binding one; by homogeneity the
    exercise region only shrinks with k), minus `safety` columns for f32
    boundary fuzz on the device."""
    nb = len(kmin_b)
    kk = kmin_b[:, None]
    v = np.maximum(kk - _S_TERM[None, :], 0.0)
    lo = np.zeros((nb, N), dtype=np.int64)
    cur = np.full(nb, 1 << 30, dtype=np.int64)
    sb0 = _s_base(np.arange(N))
    for t in range(N):
        w = np.minimum(N - t, cap_b)
        pay = kk - (C_ ** t) * sb0[None, :]
        cont = W0C * v[:, :-1] + W1C * v[:, 1:]
        exw = pay >= cont
        pref = np.zeros(nb, dtype=np.int64)
        for b in range(nb):
            e = exw[b, : w[b]]
            pref[b] = w[b] if e.all() else int(np.argmin(e))
        lot = np.maximum(0, np.minimum.reduce([cur, pref - safety, w - 1]))
        lo[:, t] = lot
        cur = lot
        v = np.concatenate([np.maximum(cont, pay), v[:, -1:]], axis=1)
    return lo


def _opt_shifts(lo: np.ndarray, ub0: np.ndarray):
    """Integer shifts s_b >= 0 minimizing sum_t window width. The
    objective sum_t[max_b(ub+s) - min_b(lo+s)] is convex in s (sum of
    max of affine); coordinate descent from top-alignment."""
    s = (ub0[:, 0].max() - ub0[:, 0]).astype(np.int64)

    def total(sv):
        u = (ub0 + sv[:, None]).max(axis=0)
        l = (lo + sv[:, None]).min(axis=0)
        return int((u - l).sum())

    best = total(s)
    for _sweep in range(8):
        improved = False
        for b in range(ub0.shape[0]):
            for d in (-8, -4, -2, -1, 1, 2, 4, 8):
                cand = s.copy()
                cand[b] = max(0, cand[b] + d)
                c = total(cand)
                if c < best:
                    best, s, improved = c, cand, True
        if not improved:
            break
    s -= s.min()
    return s


def _schedule(kmin_b, kmax_b):
    cap_b = np.ceil(N / 2 + np.log(np.maximum(kmax_b, 1e-6) / S0)
                    / (2.0 * SIG * SQRT_DT)).astype(np.int64) + 2
    cap_b = np.clip(cap_b, 1, N)
    lo = _block_lo(kmin_b, cap_b)
    t = np.arange(N)
    # Upper edge: exact-zero support is min(cap, N-t), but values decay to
    # << tolerance within ~1.5*sqrt(t)+4 columns above the (drifting) ATM
    # point, so truncate there. Stale columns above the window hold the
    # tiny value from when the edge passed them; emulator-verified rel err
    # 3e-7 (tolerance 2e-2).
    jatm = ((N - 1 - t[None, :]) / 2.0
            + (np.log(np.maximum(kmax_b, 1e-6) / S0)
               / (2.0 * SIG * SQRT_DT))[:, None])
    marg = np.ceil(1.5 * np.sqrt(np.maximum(t, 1)) + 4.0).astype(np.int64)
    ub0 = np.minimum(np.minimum(cap_b[:, None], N - t[None, :]),
                     np.ceil(jatm).astype(np.int64) + marg[None, :])
    ub0 = np.maximum(ub0, 1)
    s = _opt_shifts(lo, ub0)
    ub = (ub0 + s[:, None]).max(axis=0)
    lo_s = (lo + s[:, None]).min(axis=0)
    A = np.minimum.accumulate(np.maximum(lo_s, 0))   # monotone: -inf prefix
    A = np.minimum(A, ub - 1)
    m = t >> 1
    return {
        "cap": cap_b, "lo": lo, "s": s,
        "A": A.astype(int), "B": ub.astype(int),
        "P": int(ub.max()) + 2, "PW": int((m + ub).max()) + 1,
        "SM": int(s.max()),
    }


_cache: dict = {}


def _build(sched, reps: int = 1, hwloop: bool = False):
    """Build + compile the Bass program (single DVE stream, all groups)."""
    import concourse.bacc as bacc
    import concourse.mybir as mybir
    import concourse.tile as tile

    A, Bw = sched["A"], sched["B"]
    P, PW, SM = sched["P"], sched["PW"], sched["SM"]
    f32 = mybir.dt.float32
    nc = bacc.Bacc("TRN2", target_bir_lowering=False, debug=False,
                   num_devices=NCORES)
    v0d = nc.dram_tensor("v0", [NPART, NG, P], f32, kind="ExternalInput")
    p0d = nc.dram_tensor("pay0", [NPART, NG, PW], f32, kind="ExternalInput")
    p1d = nc.dram_tensor("pay1", [NPART, NG, PW], f32, kind="ExternalInput")
    outd = nc.dram_tensor("out", [NPART, NG, SM + 1], f32,
                          kind="ExternalOutput")

    mult = mybir.AluOpType.mult
    add = mybir.AluOpType.add
    amax = mybir.AluOpType.max

    with tile.TileContext(nc) as tc:
        with tc.tile_pool(name="state", bufs=1) as pool:
            V = pool.tile([NPART, NG, P], f32, name="V")
            U = pool.tile([NPART, NG, P], f32, name="U")
            P0 = pool.tile([NPART, NG, PW], f32, name="P0")
            P1 = pool.tile([NPART, NG, PW], f32, name="P1")
            nc.sync.dma_start(P0[:], p0d[:])
            nc.sync.dma_start(P1[:], p1d[:])

            def body():
                nc.sync.dma_start(V[:], v0d[:])
                for t in range(N):
                    a, b = int(A[t]), int(Bw[t])
                    m = t >> 1
                    pay = P0 if (t & 1) == 0 else P1
                    nc.vector.scalar_tensor_tensor(
                        U[:, :, a:b], V[:, :, a + 1:b + 1], RRATIO,
                        V[:, :, a:b], mult, add)
                    nc.vector.scalar_tensor_tensor(
                        V[:, :, a:b], U[:, :, a:b], W0C,
                        pay[:, :, m + a:m + b], mult, amax)

            if hwloop and reps > 1:
                with tc.For_i(0, reps):
                    body()
            else:
                for _ in range(reps):
                    body()

            nc.sync.dma_start(outd[:], V[:, :, 0:SM + 1])

    nc.compile()
    return nc


def build_timing(sched, reps: int):
    """Program with the kernel body repeated `reps` times in an on-device
    hardware loop — for wall-clock slope timing (used by test.py)."""
    return _build(sched, reps=reps, hwloop=True)


def _prep_inputs_v3(ks_blocks, sched):
    """Per-core input dicts from sorted strike blocks (NBLK, NPART)."""
    P, PW = sched["P"], sched["PW"]
    s_b, lo, cap = sched["s"], sched["lo"], sched["cap"]
    in_maps = []
    i_idx = np.arange(P)
    x_idx = np.arange(PW)
    for c in range(NCORES):
        v0 = np.zeros((NPART, NG, P), np.float32)
        pay0 = np.zeros((NPART, NG, PW), np.float32)
        pay1 = np.zeros((NPART, NG, PW), np.float32)
        for g in range(NG):
            blk = c * NG + g
            kk = ks_blocks[blk].astype(np.float64)[:, None]
            s = int(s_b[blk])
            j = i_idx - s
            st = np.full(P, np.inf)
            valid = (j >= 0) & (j <= N)
            st[valid] = _S_TERM[j[valid]]
            row = np.maximum(kk - st[None, :], 0.0)
            row[:, i_idx > int(cap[blk]) + s] = 0.0
            row[:, i_idx < int(lo[blk, 0]) + s] = -np.inf
            v0[:, g, :] = row.astype(np.float32)
            sb = _s_base(x_idx - s)
            pay0[:, g, :] = (kk - sb[None, :]).astype(np.float32)
            pay1[:, g, :] = (kk - (C_ * sb)[None, :]).astype(np.float32)
        in_maps.append({"v0": v0, "pay0": pay0, "pay1": pay1})
    return in_maps


def _gather_out(res_results, sched, perm):
    s_b = sched["s"]
    vals_sorted = np.empty(B, np.float64)
    for c in range(NCORES):
        o = res_results[c]["out"]
        for g in range(NG):
            blk = c * NG + g
            vals_sorted[blk * NPART:(blk + 1) * NPART] = o[:, g, int(s_b[blk])]
    out = np.empty(B, np.float64)
    out[perm] = vals_sorted
    return out


def prepare(k: np.ndarray):
    k_flat = np.asarray(k, dtype=np.float32).reshape(B).astype(np.float64)
    perm = np.argsort(k_flat, kind="stable")
    ks_blocks = k_flat[perm].reshape(NBLK, NPART)
    sched = _schedule(ks_blocks.min(axis=1), ks_blocks.max(axis=1))
    return ks_blocks, sched, perm


def _run(k: np.ndarray, trace: bool = False):
    from concourse.bass_utils import run_bass_kernel_spmd

    ks_blocks, sched, perm = prepare(k)
    key = (tuple(sched["A"]), tuple(sched["B"]), sched["P"], sched["PW"],
           sched["SM"])
    if key not in _cache:
        _cache[key] = _build(sched)
    nc = _cache[key]
    in_maps = _prep_inputs_v3(ks_blocks, sched)
    res = run_bass_kernel_spmd(nc, in_maps, core_ids=list(range(NCORES)),
                               trace=trace)
    out = _gather_out(res.results, sched, perm)
    return out.astype(np.float32).reshape(B, 1), res


def kernel(k: np.ndarray) -> np.ndarray:
    out, _ = _run(k)
    return out
